# revision 31
# baseline (speedup 1.0000x reference)
"""Trainium2 Bass kernel for causal multi-head attention.

Problem: x[4, 2048, 1024] -> Attention(heads=16, causal) -> out[4, 2048, 1024]

Sharding over 8 NeuronCores: core c handles batch bi = c // 2 and head-half
hj = c % 2 (8 of the 16 heads).  Each core computes its 8 heads' attention
and a partial output projection (row-parallel Wo); the host sums the two
partials per batch element and adds bo (the all-reduce step).

Per-core kernel (n=2048 tokens, dloc=512 local features, dh=64, 8 heads):
  - Host supplies x^T in fp16 (contraction dim on SBUF partitions, no
    on-device transposes).
  - Q^T, K^T [128 feats (pair of heads), 2048] per head-pair; V [2048, 512]
    in natural layout with a ones-column per head (V' = [V | 1]) so the
    PV matmul accumulates softmax denominators for free.
  - Scores are computed transposed: S^T[j, i] = k_j . q_i with K^T slices
    as the stationary operand.  K = dh = 64, and the two heads of a pair
    live at partition bases 0 / 64, which maps to PE row-groups 0-1 / 2-3:
    the hardware runs the pair concurrently (row packing).
  - exp on ScalarE straight out of PSUM (scale = 1/8 fused into the
    activation); causal mask applied post-exp by gpsimd affine_select
    (fill 0) on diagonal tiles only.
  - O'^T[f, i] accumulated over j-tiles in PSUM with lhsT = V'; row 64 is
    the softmax denominator.  O' is immediately staged PSUM->SBUF (frees
    the accumulator bank), then normalized off the critical path:
    1/sum via ln -> exp(-x) on ScalarE, broadcast down partitions with a
    K=1 outer-product matmul, multiply + bias on VectorE.
  - Output projection contracts the 512 local features from O^T directly.

All matmul operand tensors are float16 (fp32 accumulation in PSUM).  fp16
streams at 1 cycle/row like bf16 and allows standalone LDWEIGHTS (fp32/f32r
matmuls embed the weight load and serialize it, ~+150ns per matmul), but
keeps an 11-bit mantissa: end-to-end error vs the fp32 reference is ~5e-4.

Softmax reciprocals: the 1024 per-chunk denominators are DMA-repacked onto
128 partitions, reciprocal'd in one cheap DVE op ([1, N] DVE reciprocal is
~6.4 cyc/elem/lane, i.e. ~3.3us per row), DMA'd back to a partition-0 row
and broadcast down 64 partitions with the gpsimd partition_broadcast custom
instruction (which reads physical partition 0).
"""

import os
import sys

for _p in ("/opt/trn_rl_repo",):
    if _p not in sys.path and os.path.isdir(_p):
        sys.path.insert(0, _p)

import numpy as np

import concourse.bass as bass
import concourse.mybir as mybir
import concourse.tile as tile
from concourse import bacc
from concourse import bass_utils

ts = bass.ts
F32 = mybir.dt.float32
F16 = mybir.dt.float16

P = 128          # SBUF partitions
N = 2048         # sequence length
D = 1024         # model dim
DLOC = 512       # local (per-core) feature dim = 8 heads * 64
DH = 64          # head dim
NPAIR = 4        # head pairs per core (2 heads per pair = 128 feats)
NCO = D // P     # 8 contraction tiles over model dim
NJT = N // P     # 16 key tiles of 128
NCH = N // 512   # 4 query chunks of 512
SCALE = DH ** -0.5


def _emit_kernel(tc, xT_d, wq_d, wk_d, wv_d, wo_d, bq_d, bk_d, bv_d, out_d):
    nc = tc.nc
    EXP = mybir.ActivationFunctionType.Exp
    GE = mybir.AluOpType.is_ge

    xTr = xT_d.rearrange("(o p) t -> p o t", p=P)
    wqr = wq_d.rearrange("(o p) f -> p o f", p=P)
    wkr = wk_d.rearrange("(o p) f -> p o f", p=P)
    wvr = wv_d.rearrange("(o p) f -> p o f", p=P)
    wor = wo_d.rearrange("(o p) e -> p o e", p=P)

    with (
        nc.allow_low_precision(reason="fp16 operands / fp32 accumulation"),
        tc.tile_pool(name="perm", bufs=1) as perm,
        tc.tile_pool(name="shared", bufs=1) as shared,
        tc.tile_pool(name="qkt", bufs=3) as qktp,
        tc.tile_pool(name="pexp", bufs=6) as pexp,
        tc.tile_pool(name="stg", bufs=4) as stgp,
        tc.tile_pool(name="rrp", bufs=3) as rrp,
        tc.tile_pool(name="outsb", bufs=3) as outsb,
        tc.tile_pool(name="psS", bufs=2, space="PSUM") as psS,
        tc.tile_pool(name="psO", bufs=1, space="PSUM") as psO,
        tc.tile_pool(name="psProj", bufs=2, space="PSUM") as psProj,
    ):
        # ---- constants / weights ----------------------------------------
        bq_sb = perm.tile([P, NPAIR], F32, name="bq_sb")
        bk_sb = perm.tile([P, NPAIR], F32, name="bk_sb")
        bv_sb = perm.tile([P, NPAIR], F32, name="bv_sb")

        # V' = [V | 1] per head: [128 j, jt, head, 65] fp16 (fp16 memset is
        # ISA-legal, unlike f32r; a broadcast DMA here would be 16K 2-byte
        # descriptors = ~150us of queue serialization)
        Vp = perm.tile([P, NJT, 8, DH + 1], F16, name="Vp")
        nc.vector.memset(Vp[:, :, :, DH:], 1.0)

        # DMA order matters: the first projection matmuls need wq + xT chunk
        # 0, so issue those first and the rest behind them.
        xT_sb = perm.tile([P, NCO, N], F16, name="xT_sb")
        wq_sb = shared.tile([P, NCO, DLOC], F16, name="wq_sb", tag="wq_wo")
        wk_sb = shared.tile([P, NCO, DLOC], F16, name="wk_sb", tag="wk")
        wv_sb = shared.tile([P, NCO, DLOC], F16, name="wv_sb", tag="wv_ot")
        for co in range(NCO):
            nc.sync.dma_start(out=wq_sb[:, co, :], in_=wqr[:, co, :])
            nc.sync.dma_start(
                out=xT_sb[:, co, ts(0, 512)], in_=xTr[:, co, ts(0, 512)]
            )
        for ch in range(1, NCH):
            for co in range(NCO):
                nc.sync.dma_start(
                    out=xT_sb[:, co, ts(ch, 512)], in_=xTr[:, co, ts(ch, 512)]
                )
        for co in range(NCO):
            nc.sync.dma_start(out=wk_sb[:, co, :], in_=wkr[:, co, :])
        for co in range(NCO):
            nc.sync.dma_start(out=wv_sb[:, co, :], in_=wvr[:, co, :])
        nc.sync.dma_start(out=bq_sb, in_=bq_d)
        nc.sync.dma_start(out=bk_sb, in_=bk_d)
        nc.sync.dma_start(out=bv_sb, in_=bv_d)

        qk_tiles = {}

        def qkproj_gen(pair, use_big_psum):
            """Emit Q^T / K^T projection for one head pair; yields between ops."""
            QT = qktp.tile([P, N], F16, name=f"QT{pair}", tag="qt")
            KT = qktp.tile([P, N], F16, name=f"KT{pair}", tag="kt")
            qk_tiles[pair] = (QT, KT)
            for wsb, dst, bias in ((wq_sb, QT, bq_sb), (wk_sb, KT, bk_sb)):
                for ch in range(NCH):
                    if use_big_psum:
                        grp = psS.tile([P, 2, 512], F32, name="pj", tag="sg")
                        acc = grp[:, 0, :]
                    else:
                        acc = psProj.tile([P, 512], F32, name="pj", tag="proj")
                    for co in range(NCO):
                        nc.tensor.matmul(
                            acc,
                            lhsT=wsb[:, co, ts(pair, P)],
                            rhs=xT_sb[:, co, ts(ch, 512)],
                            start=(co == 0),
                            stop=(co == NCO - 1),
                        )
                        yield
                    nc.vector.tensor_scalar_add(
                        out=dst[:, ts(ch, 512)],
                        in0=acc,
                        scalar1=bias[:, pair : pair + 1],
                    )
                    yield "end"

        def vproj_gen(jt0, jt1, use_big_psum):
            for jt in range(jt0, jt1):
                if use_big_psum:
                    grp = psS.tile([P, 2, 512], F32, name="vps", tag="sg")
                    acc = grp[:, 0, :]
                else:
                    acc = psProj.tile([P, 512], F32, name="vps", tag="proj")
                for co in range(NCO):
                    nc.tensor.matmul(
                        acc,
                        lhsT=xT_sb[:, co, ts(jt, P)],
                        rhs=wv_sb[:, co, :],
                        start=(co == 0),
                        stop=(co == NCO - 1),
                    )
                    yield
                nc.vector.tensor_copy(
                    out=Vp[:, jt, :, 0:DH],
                    in_=acc.rearrange("p (h f) -> p h f", h=8),
                )
                yield "end"

        def chain(*gens):
            for g in gens:
                yield from g

        class Fill:
            """Dispenses filler ops; a PSUM-accumulator group must never
            straddle an attention chunk boundary (its DVE eviction would
            queue behind the next chunk's PV start while PV waits on the
            slot that eviction frees -> deadlock)."""

            def __init__(self, gen):
                self.gen = gen
                self.in_group = False

            def _next(self):
                v = next(self.gen, StopIteration)
                if v is StopIteration:
                    self.gen = None
                    self.in_group = False
                    return False
                self.in_group = v != "end"
                return True

            def pull(self, n):
                for _ in range(n):
                    if self.gen is None or not self._next():
                        return

            def finish_group(self):
                while self.gen is not None and self.in_group:
                    self._next()

            def drain(self):
                while self.gen is not None and self._next():
                    pass

        def attn_emit(pair, fill, after_chunk=None):
            QT, KT = qk_tiles[pair]
            hA, hB = 2 * pair, 2 * pair + 1
            # big chunk first: the last chunk's normalize chain then hides
            # under the previous chunk's output-projection matmuls
            chunk_order = list(range(NCH - 1, -1, -1))
            for chi, ch in enumerate(chunk_order):
                if after_chunk is not None and chi > 0:
                    after_chunk(chunk_order[chi - 1])
                oA = psO.tile([P, 512], F32, name="oA", tag="oA")
                oB = psO.tile([P, 512], F32, name="oB", tag="oB")
                njt = 4 * ch + 4
                prev = None

                def pv(pt, jt, njt=njt, oA=oA, oB=oB, hA=hA, hB=hB, ch=ch):
                    # below-diagonal columns of pt are all-zero: skip them
                    # (they contribute nothing; has_written tracking is
                    # per-element so partial-width accumulation is fine, and
                    # jt==0 always writes the full width)
                    plo = P * (jt - 4 * ch) if jt - 4 * ch > 0 else 0
                    for h01, (oP, h) in enumerate(((oA, hA), (oB, hB))):
                        nc.tensor.matmul(
                            oP[0 : DH + 1, plo:512],
                            lhsT=Vp[:, jt, h, :],
                            rhs=pt[:, h01, plo:512],
                            start=(jt == 0),
                            stop=(jt == njt - 1),
                        )

                for jt in range(njt):
                    if fill is not None:
                        fill.pull(2)
                    sg = psS.tile([P, 2, 512], F32, name="sg", tag="sg")
                    # diagonal j-tiles: columns q < 128*r are entirely below
                    # the causal diagonal; skip computing them (the masking
                    # affine_select fills that region of pt with 0 anyway,
                    # covering the garbage left in PSUM)
                    r0 = jt - 4 * ch
                    lo = P * r0 if r0 > 0 else 0
                    nc.tensor.matmul(
                        sg[:, 0, lo:512],
                        lhsT=KT[0:DH, ts(jt, P)],
                        rhs=QT[0:DH, 512 * ch + lo : 512 * (ch + 1)],
                        start=True,
                        stop=True,
                    )
                    nc.tensor.matmul(
                        sg[:, 1, lo:512],
                        lhsT=KT[DH:P, ts(jt, P)],
                        rhs=QT[DH:P, 512 * ch + lo : 512 * (ch + 1)],
                        start=True,
                        stop=True,
                    )
                    pt = pexp.tile([P, 2, 512], F16, name="pt", tag="pt")
                    nc.scalar.activation(
                        out=pt[:, :, lo:512], in_=sg[:, :, lo:512], func=EXP,
                        scale=SCALE,
                    )
                    r = jt - 4 * ch
                    if r >= 0:
                        if lo > 0:
                            # columns entirely below the diagonal: never
                            # computed, but read by the PV matmul -> zero them
                            nc.gpsimd.memset(pt[:, :, 0:lo], 0.0)
                        for h01 in (0, 1):
                            # keep where q >= 128*r + p  (j_global <= i_global)
                            nc.gpsimd.affine_select(
                                out=pt[:, h01, lo : lo + P],
                                in_=pt[:, h01, lo : lo + P],
                                compare_op=GE,
                                fill=0.0,
                                base=0,
                                channel_multiplier=-1,
                                pattern=[[1, P]],
                            )
                    if prev is not None:
                        pv(*prev)
                    prev = (pt, jt)
                pv(*prev)
                if fill is not None:
                    fill.finish_group()

                # ---- stage O' out of PSUM, then normalize off-path --------
                st = stgp.tile([DH + 1, 2, 512], F32, name="st", tag="st")
                nc.vector.tensor_copy(out=st[:, 0, :], in_=oA[0 : DH + 1, :])
                nc.vector.tensor_copy(out=st[:, 1, :], in_=oB[0 : DH + 1, :])
                # Reciprocal of the 1024 sums at full DVE lane utilization:
                # DVE recip costs ~6.4 cyc/elem PER LANE, so [1, 1024] would
                # be ~7us.  Pack the sums onto 128 partitions via DMA
                # (element-wise stream repack), recip [128, 8] (~0.1us),
                # unpack to a partition-0 row, then gpsimd-broadcast it down
                # 64 partitions (partition_broadcast reads physical p0).
                pk = rrp.tile([P, 8], F32, name="pk", tag="pk")
                rrow = rrp.tile([1, 2, 512], F32, name="rrow", tag="rrow")
                Rs = rrp.tile([DH, 2, 512], F32, name="Rs", tag="Rs")
                nc.sync.dma_start(out=pk, in_=st[DH : DH + 1, :, :])
                nc.vector.reciprocal(out=pk, in_=pk)
                nc.sync.dma_start(out=rrow, in_=pk)
                for h01 in (0, 1):
                    nc.gpsimd.partition_broadcast(Rs[:, h01, :], rrow[0:1, h01, :])
                nc.vector.tensor_mul(
                    out=OT[0:DH, pair, ts(ch, 512)], in0=st[0:DH, 0, :], in1=Rs[:, 0, :]
                )
                nc.vector.tensor_scalar_add(
                    out=OT[0:DH, pair, ts(ch, 512)],
                    in0=OT[0:DH, pair, ts(ch, 512)],
                    scalar1=bv_sb[0:DH, pair : pair + 1],
                )
                nc.vector.tensor_mul(
                    out=OT[DH:P, pair, ts(ch, 512)], in0=st[0:DH, 1, :], in1=Rs[:, 1, :]
                )
                nc.vector.tensor_scalar_add(
                    out=OT[DH:P, pair, ts(ch, 512)],
                    in0=OT[DH:P, pair, ts(ch, 512)],
                    scalar1=bv_sb[DH:P, pair : pair + 1],
                )
                if after_chunk is not None and chi == NCH - 1:
                    after_chunk(ch)
            if fill is not None:
                fill.drain()

        def outproj_chunk(ch):
            for it in range(4 * ch, 4 * ch + 4):
                for e in range(2):
                    acc = psProj.tile([P, 512], F32, name="ops", tag="proj")
                    for p4 in range(NPAIR):
                        nc.tensor.matmul(
                            acc,
                            lhsT=OT[:, p4, ts(it, P)],
                            rhs=wo_sb[:, p4, ts(e, 512)],
                            start=(p4 == 0),
                            stop=(p4 == NPAIR - 1),
                        )
                    ob = outsb.tile([P, 512], F32, name="ob", tag="ob")
                    nc.vector.tensor_copy(out=ob, in_=acc)
                    nc.sync.dma_start(out=out_d[ts(it, P), ts(e, 512)], in_=ob)

        # ---- emission schedule ------------------------------------------
        # upfront: QK projections for pair 0 + all of V (attention consumes
        # V j-tiles far faster than a fill stream could produce them)
        for _ in qkproj_gen(0, use_big_psum=True):
            pass
        for _ in vproj_gen(0, NJT, use_big_psum=True):
            pass

        # OT reuses wv's slot (wv dead once V is projected)
        OT = shared.tile([P, NPAIR, N], F16, name="OT", tag="wv_ot")

        def primed(gen):
            next(gen)
            return gen

        # attention for pair p overlapped with projections for pair p+1
        attn_emit(0, Fill(primed(qkproj_gen(1, use_big_psum=False))))
        attn_emit(1, Fill(primed(qkproj_gen(2, use_big_psum=False))))
        attn_emit(2, Fill(primed(qkproj_gen(3, use_big_psum=False))))

        # wo reuses wq's slot (wq dead after pair-3 projections)
        wo_sb = shared.tile([P, NPAIR, D], F16, name="wo_sb", tag="wq_wo")
        for o4 in range(NPAIR):
            nc.sync.dma_start(out=wo_sb[:, o4, :], in_=wor[:, o4, :])

        # pair 3 with the output projection interleaved per finished chunk
        attn_emit(3, None, after_chunk=outproj_chunk)


def build():
    nc = bacc.Bacc("TRN2", target_bir_lowering=False, debug=False, num_devices=8)
    xT_d = nc.dram_tensor("xT", [D, N], F16, kind="ExternalInput").ap()
    wq_d = nc.dram_tensor("wq", [D, DLOC], F16, kind="ExternalInput").ap()
    wk_d = nc.dram_tensor("wk", [D, DLOC], F16, kind="ExternalInput").ap()
    wv_d = nc.dram_tensor("wv", [D, DLOC], F16, kind="ExternalInput").ap()
    wo_d = nc.dram_tensor("wo", [DLOC, D], F16, kind="ExternalInput").ap()
    bq_d = nc.dram_tensor("bq", [P, NPAIR], F32, kind="ExternalInput").ap()
    bk_d = nc.dram_tensor("bk", [P, NPAIR], F32, kind="ExternalInput").ap()
    bv_d = nc.dram_tensor("bv", [P, NPAIR], F32, kind="ExternalInput").ap()
    out_d = nc.dram_tensor("out", [N, D], F32, kind="ExternalOutput").ap()
    with tile.TileContext(nc) as tc:
        _emit_kernel(tc, xT_d, wq_d, wk_d, wv_d, wo_d, bq_d, bk_d, bv_d, out_d)
    nc.compile()
    return nc


_NC = None


def _get_nc():
    global _NC
    if _NC is None:
        _NC = build()
    return _NC


def make_in_maps(x, Wq, bq, Wkv, bkv, Wo, bo):
    x = np.asarray(x, dtype=np.float32)
    Wq = np.asarray(Wq, dtype=np.float32)
    bq = np.asarray(bq, dtype=np.float32)
    Wkv = np.asarray(Wkv, dtype=np.float32)
    bkv = np.asarray(bkv, dtype=np.float32)
    Wo = np.asarray(Wo, dtype=np.float32)

    in_maps = []
    for c in range(8):
        bi, hj = c // 2, c % 2
        sl = slice(hj * DLOC, (hj + 1) * DLOC)
        slv = slice(D + hj * DLOC, D + (hj + 1) * DLOC)
        in_maps.append(
            {
                "xT": np.ascontiguousarray(x[bi].T).astype(np.float16),
                "wq": np.ascontiguousarray(Wq[:, sl]).astype(np.float16),
                "wk": np.ascontiguousarray(Wkv[:, sl]).astype(np.float16),
                "wv": np.ascontiguousarray(Wkv[:, slv]).astype(np.float16),
                "wo": np.ascontiguousarray(Wo[sl, :]).astype(np.float16),
                "bq": np.ascontiguousarray(bq[sl].reshape(NPAIR, P).T),
                "bk": np.ascontiguousarray(bkv[sl].reshape(NPAIR, P).T),
                "bv": np.ascontiguousarray(bkv[slv].reshape(NPAIR, P).T),
            }
        )
    return in_maps


def combine_outputs(results, bo):
    bo = np.asarray(bo, dtype=np.float32)
    outs = [results[c]["out"] for c in range(8)]
    full = np.stack([outs[2 * bi] + outs[2 * bi + 1] for bi in range(4)])
    return (full + bo[None, None, :]).astype(np.float32)


def kernel(x, Wq, bq, Wkv, bkv, Wo, bo, **_ignored):
    nc = _get_nc()
    in_maps = make_in_maps(x, Wq, bq, Wkv, bkv, Wo, bo)
    res = bass_utils.run_bass_kernel_spmd(nc, in_maps, core_ids=list(range(8)))
    return combine_outputs(res.results, bo)


# revision 32
# speedup vs baseline: 1.0450x; 1.0450x over previous
"""Trainium2 Bass kernel for causal multi-head attention.

Problem: x[4, 2048, 1024] -> Attention(heads=16, causal) -> out[4, 2048, 1024]

Sharding over 8 NeuronCores: core c handles batch bi = c // 2 and head-half
hj = c % 2 (8 of the 16 heads).  Each core computes its 8 heads' attention
and a partial output projection (row-parallel Wo); the host sums the two
partials per batch element and adds bo (the all-reduce step).

Per-core kernel (n=2048 tokens, dloc=512 local features, dh=64, 8 heads):
  - Host supplies x^T in fp16 (contraction dim on SBUF partitions, no
    on-device transposes).
  - Q^T, K^T [128 feats (pair of heads), 2048] per head-pair; V [2048, 512]
    in natural layout with a ones-column per head (V' = [V | 1]) so the
    PV matmul accumulates softmax denominators for free.
  - Scores are computed transposed: S^T[j, i] = k_j . q_i with K^T slices
    as the stationary operand.  K = dh = 64, and the two heads of a pair
    live at partition bases 0 / 64, which maps to PE row-groups 0-1 / 2-3:
    the hardware runs the pair concurrently (row packing).
  - exp on ScalarE straight out of PSUM (scale = 1/8 fused into the
    activation); causal mask applied post-exp by gpsimd affine_select
    (fill 0) on diagonal tiles only.
  - O'^T[f, i] accumulated over j-tiles in PSUM with lhsT = V'; row 64 is
    the softmax denominator.  O' is immediately staged PSUM->SBUF (frees
    the accumulator bank), then normalized off the critical path:
    1/sum via ln -> exp(-x) on ScalarE, broadcast down partitions with a
    K=1 outer-product matmul, multiply + bias on VectorE.
  - Output projection contracts the 512 local features from O^T directly.

All matmul operand tensors are float16 (fp32 accumulation in PSUM).  fp16
streams at 1 cycle/row like bf16 and allows standalone LDWEIGHTS (fp32/f32r
matmuls embed the weight load and serialize it, ~+150ns per matmul), but
keeps an 11-bit mantissa: end-to-end error vs the fp32 reference is ~5e-4.

Softmax reciprocals: the 1024 per-chunk denominators are DMA-repacked onto
128 partitions, reciprocal'd in one cheap DVE op ([1, N] DVE reciprocal is
~6.4 cyc/elem/lane, i.e. ~3.3us per row), DMA'd back to a partition-0 row
and broadcast down 64 partitions with the gpsimd partition_broadcast custom
instruction (which reads physical partition 0).
"""

import os
import sys

for _p in ("/opt/trn_rl_repo",):
    if _p not in sys.path and os.path.isdir(_p):
        sys.path.insert(0, _p)

import numpy as np

import concourse.bass as bass
import concourse.mybir as mybir
import concourse.tile as tile
from concourse import bacc
from concourse import bass_utils

ts = bass.ts
F32 = mybir.dt.float32
F16 = mybir.dt.float16

P = 128          # SBUF partitions
N = 2048         # sequence length
D = 1024         # model dim
DLOC = 512       # local (per-core) feature dim = 8 heads * 64
DH = 64          # head dim
NPAIR = 4        # head pairs per core (2 heads per pair = 128 feats)
NCO = D // P     # 8 contraction tiles over model dim
NJT = N // P     # 16 key tiles of 128
NCH = N // 512   # 4 query chunks of 512
SCALE = DH ** -0.5


def _emit_kernel(tc, xT_d, wq_d, wk_d, wv_d, wo_d, bq_d, bk_d, bv_d, out_d):
    nc = tc.nc
    EXP = mybir.ActivationFunctionType.Exp
    GE = mybir.AluOpType.is_ge

    xTr = xT_d.rearrange("(o p) t -> p o t", p=P)
    wqr = wq_d.rearrange("(o p) f -> p o f", p=P)
    wkr = wk_d.rearrange("(o p) f -> p o f", p=P)
    wvr = wv_d.rearrange("(o p) f -> p o f", p=P)
    wor = wo_d.rearrange("(o p) e -> p o e", p=P)

    with (
        nc.allow_low_precision(reason="fp16 operands / fp32 accumulation"),
        tc.tile_pool(name="perm", bufs=1) as perm,
        tc.tile_pool(name="shared", bufs=1) as shared,
        tc.tile_pool(name="qkt", bufs=3) as qktp,
        tc.tile_pool(name="pexp", bufs=6) as pexp,
        tc.tile_pool(name="stg", bufs=4) as stgp,
        tc.tile_pool(name="rrp", bufs=3) as rrp,
        tc.tile_pool(name="outsb", bufs=3) as outsb,
        tc.tile_pool(name="psS", bufs=2, space="PSUM") as psS,
        tc.tile_pool(name="psO", bufs=1, space="PSUM") as psO,
        tc.tile_pool(name="psProj", bufs=2, space="PSUM") as psProj,
    ):
        # ---- constants / weights ----------------------------------------
        bq_sb = perm.tile([P, NPAIR], F32, name="bq_sb")
        bk_sb = perm.tile([P, NPAIR], F32, name="bk_sb")
        bv_sb = perm.tile([P, NPAIR], F32, name="bv_sb")
        nc.sync.dma_start(out=bq_sb, in_=bq_d)
        nc.sync.dma_start(out=bk_sb, in_=bk_d)
        nc.sync.dma_start(out=bv_sb, in_=bv_d)

        # V' = [V | 1] per head: [128 j, jt, head, 65] fp16 (fp16 memset is
        # ISA-legal, unlike f32r; a broadcast DMA here would be 16K 2-byte
        # descriptors = ~150us of queue serialization)
        Vp = perm.tile([P, NJT, 8, DH + 1], F16, name="Vp")
        nc.vector.memset(Vp[:, :, :, DH:], 1.0)

        # DMA order matters: the first projection matmuls need wq + xT chunk
        # 0, so issue those first and the rest behind them.
        xT_sb = perm.tile([P, NCO, N], F16, name="xT_sb")
        wq_sb = shared.tile([P, NCO, DLOC], F16, name="wq_sb", tag="wq_wo")
        wk_sb = shared.tile([P, NCO, DLOC], F16, name="wk_sb", tag="wk")
        wv_sb = shared.tile([P, NCO, DLOC], F16, name="wv_sb", tag="wv_ot")
        for co in range(NCO):
            nc.sync.dma_start(out=wq_sb[:, co, :], in_=wqr[:, co, :])
            nc.sync.dma_start(
                out=xT_sb[:, co, ts(0, 512)], in_=xTr[:, co, ts(0, 512)]
            )
        for ch in range(1, NCH):
            for co in range(NCO):
                nc.sync.dma_start(
                    out=xT_sb[:, co, ts(ch, 512)], in_=xTr[:, co, ts(ch, 512)]
                )
        for co in range(NCO):
            nc.sync.dma_start(out=wk_sb[:, co, :], in_=wkr[:, co, :])
        for co in range(NCO):
            nc.sync.dma_start(out=wv_sb[:, co, :], in_=wvr[:, co, :])

        qk_tiles = {}

        def qkproj_gen(pair, use_big_psum):
            """Emit Q^T / K^T projection for one head pair; yields between ops."""
            QT = qktp.tile([P, N], F16, name=f"QT{pair}", tag="qt")
            KT = qktp.tile([P, N], F16, name=f"KT{pair}", tag="kt")
            qk_tiles[pair] = (QT, KT)
            for wsb, dst, bias in ((wq_sb, QT, bq_sb), (wk_sb, KT, bk_sb)):
                for ch in range(NCH):
                    if use_big_psum:
                        grp = psS.tile([P, 2, 512], F32, name="pj", tag="sg")
                        acc = grp[:, 0, :]
                    else:
                        acc = psProj.tile([P, 512], F32, name="pj", tag="proj")
                    for co in range(NCO):
                        nc.tensor.matmul(
                            acc,
                            lhsT=wsb[:, co, ts(pair, P)],
                            rhs=xT_sb[:, co, ts(ch, 512)],
                            start=(co == 0),
                            stop=(co == NCO - 1),
                        )
                        yield
                    nc.vector.tensor_scalar_add(
                        out=dst[:, ts(ch, 512)],
                        in0=acc,
                        scalar1=bias[:, pair : pair + 1],
                    )
                    yield "end"

        def vproj_gen(jt0, jt1, use_big_psum):
            for jt in range(jt0, jt1):
                if use_big_psum:
                    grp = psS.tile([P, 2, 512], F32, name="vps", tag="sg")
                    acc = grp[:, 0, :]
                else:
                    acc = psProj.tile([P, 512], F32, name="vps", tag="proj")
                for co in range(NCO):
                    nc.tensor.matmul(
                        acc,
                        lhsT=xT_sb[:, co, ts(jt, P)],
                        rhs=wv_sb[:, co, :],
                        start=(co == 0),
                        stop=(co == NCO - 1),
                    )
                    yield
                nc.vector.tensor_copy(
                    out=Vp[:, jt, :, 0:DH],
                    in_=acc.rearrange("p (h f) -> p h f", h=8),
                )
                yield "end"

        def chain(*gens):
            for g in gens:
                yield from g

        class Fill:
            """Dispenses filler ops; a PSUM-accumulator group must never
            straddle an attention chunk boundary (its DVE eviction would
            queue behind the next chunk's PV start while PV waits on the
            slot that eviction frees -> deadlock)."""

            def __init__(self, gen):
                self.gen = gen
                self.in_group = False

            def _next(self):
                v = next(self.gen, StopIteration)
                if v is StopIteration:
                    self.gen = None
                    self.in_group = False
                    return False
                self.in_group = v != "end"
                return True

            def pull(self, n):
                for _ in range(n):
                    if self.gen is None or not self._next():
                        return

            def finish_group(self):
                while self.gen is not None and self.in_group:
                    self._next()

            def drain(self):
                while self.gen is not None and self._next():
                    pass

        def attn_emit(pair, fill, after_chunk=None):
            QT, KT = qk_tiles[pair]
            hA, hB = 2 * pair, 2 * pair + 1
            # big chunk first: the last chunk's normalize chain then hides
            # under the previous chunk's output-projection matmuls
            chunk_order = list(range(NCH - 1, -1, -1))
            for chi, ch in enumerate(chunk_order):
                if after_chunk is not None and chi > 0:
                    after_chunk(chunk_order[chi - 1])
                oA = psO.tile([P, 512], F32, name="oA", tag="oA")
                oB = psO.tile([P, 512], F32, name="oB", tag="oB")
                njt = 4 * ch + 4
                prev = None

                def pv(pt, jt, njt=njt, oA=oA, oB=oB, hA=hA, hB=hB, ch=ch):
                    # below-diagonal columns of pt are all-zero: skip them
                    # (they contribute nothing; has_written tracking is
                    # per-element so partial-width accumulation is fine, and
                    # jt==0 always writes the full width)
                    plo = P * (jt - 4 * ch) if jt - 4 * ch > 0 else 0
                    for h01, (oP, h) in enumerate(((oA, hA), (oB, hB))):
                        nc.tensor.matmul(
                            oP[0 : DH + 1, plo:512],
                            lhsT=Vp[:, jt, h, :],
                            rhs=pt[:, h01, plo:512],
                            start=(jt == 0),
                            stop=(jt == njt - 1),
                        )

                for jt in range(njt):
                    if fill is not None:
                        fill.pull(2)
                    sg = psS.tile([P, 2, 512], F32, name="sg", tag="sg")
                    # diagonal j-tiles: columns q < 128*r are entirely below
                    # the causal diagonal; skip computing them (the masking
                    # affine_select fills that region of pt with 0 anyway,
                    # covering the garbage left in PSUM)
                    r0 = jt - 4 * ch
                    lo = P * r0 if r0 > 0 else 0
                    nc.tensor.matmul(
                        sg[:, 0, lo:512],
                        lhsT=KT[0:DH, ts(jt, P)],
                        rhs=QT[0:DH, 512 * ch + lo : 512 * (ch + 1)],
                        start=True,
                        stop=True,
                    )
                    nc.tensor.matmul(
                        sg[:, 1, lo:512],
                        lhsT=KT[DH:P, ts(jt, P)],
                        rhs=QT[DH:P, 512 * ch + lo : 512 * (ch + 1)],
                        start=True,
                        stop=True,
                    )
                    pt = pexp.tile([P, 2, 512], F16, name="pt", tag="pt")
                    nc.scalar.activation(
                        out=pt[:, :, lo:512], in_=sg[:, :, lo:512], func=EXP,
                        scale=SCALE,
                    )
                    r = jt - 4 * ch
                    if r >= 0:
                        if lo > 0:
                            # columns entirely below the diagonal: never
                            # computed, but read by the PV matmul -> zero them
                            nc.gpsimd.memset(pt[:, :, 0:lo], 0.0)
                        for h01 in (0, 1):
                            # keep where q >= 128*r + p  (j_global <= i_global)
                            nc.gpsimd.affine_select(
                                out=pt[:, h01, lo : lo + P],
                                in_=pt[:, h01, lo : lo + P],
                                compare_op=GE,
                                fill=0.0,
                                base=0,
                                channel_multiplier=-1,
                                pattern=[[1, P]],
                            )
                    if prev is not None:
                        pv(*prev)
                    prev = (pt, jt)
                pv(*prev)
                if fill is not None:
                    fill.finish_group()

                # ---- stage O' out of PSUM, then normalize off-path --------
                st = stgp.tile([DH + 1, 2, 512], F32, name="st", tag="st")
                nc.vector.tensor_copy(out=st[:, 0, :], in_=oA[0 : DH + 1, :])
                nc.vector.tensor_copy(out=st[:, 1, :], in_=oB[0 : DH + 1, :])
                # Reciprocal of the 1024 sums at full DVE lane utilization:
                # DVE recip costs ~6.4 cyc/elem PER LANE, so [1, 1024] would
                # be ~7us.  Pack the sums onto 128 partitions via DMA
                # (element-wise stream repack), recip [128, 8] (~0.1us),
                # unpack to a partition-0 row, then gpsimd-broadcast it down
                # 64 partitions (partition_broadcast reads physical p0).
                pk = rrp.tile([P, 8], F32, name="pk", tag="pk")
                rrow = rrp.tile([1, 2, 512], F32, name="rrow", tag="rrow")
                Rs = rrp.tile([DH, 2, 512], F32, name="Rs", tag="Rs")
                nc.sync.dma_start(out=pk, in_=st[DH : DH + 1, :, :])
                nc.vector.reciprocal(out=pk, in_=pk)
                nc.sync.dma_start(out=rrow, in_=pk)
                for h01 in (0, 1):
                    nc.gpsimd.partition_broadcast(Rs[:, h01, :], rrow[0:1, h01, :])
                nc.vector.tensor_mul(
                    out=OT[0:DH, pair, ts(ch, 512)], in0=st[0:DH, 0, :], in1=Rs[:, 0, :]
                )
                nc.vector.tensor_scalar_add(
                    out=OT[0:DH, pair, ts(ch, 512)],
                    in0=OT[0:DH, pair, ts(ch, 512)],
                    scalar1=bv_sb[0:DH, pair : pair + 1],
                )
                nc.vector.tensor_mul(
                    out=OT[DH:P, pair, ts(ch, 512)], in0=st[0:DH, 1, :], in1=Rs[:, 1, :]
                )
                nc.vector.tensor_scalar_add(
                    out=OT[DH:P, pair, ts(ch, 512)],
                    in0=OT[DH:P, pair, ts(ch, 512)],
                    scalar1=bv_sb[DH:P, pair : pair + 1],
                )
                if after_chunk is not None and chi == NCH - 1:
                    after_chunk(ch)
            if fill is not None:
                fill.drain()

        def outproj_chunk(ch):
            for it in range(4 * ch, 4 * ch + 4):
                for e in range(2):
                    acc = psProj.tile([P, 512], F32, name="ops", tag="proj")
                    for p4 in range(NPAIR):
                        nc.tensor.matmul(
                            acc,
                            lhsT=OT[:, p4, ts(it, P)],
                            rhs=wo_sb[:, p4, ts(e, 512)],
                            start=(p4 == 0),
                            stop=(p4 == NPAIR - 1),
                        )
                    ob = outsb.tile([P, 512], F32, name="ob", tag="ob")
                    nc.vector.tensor_copy(out=ob, in_=acc)
                    nc.sync.dma_start(out=out_d[ts(it, P), ts(e, 512)], in_=ob)

        # ---- emission schedule ------------------------------------------
        # upfront: QK projections for pair 0 + all of V (attention consumes
        # V j-tiles far faster than a fill stream could produce them)
        for _ in qkproj_gen(0, use_big_psum=True):
            pass
        for _ in vproj_gen(0, NJT, use_big_psum=True):
            pass

        # OT reuses wv's slot (wv dead once V is projected)
        OT = shared.tile([P, NPAIR, N], F16, name="OT", tag="wv_ot")

        def primed(gen):
            next(gen)
            return gen

        # attention for pair p overlapped with projections for pair p+1
        attn_emit(0, Fill(primed(qkproj_gen(1, use_big_psum=False))))
        attn_emit(1, Fill(primed(qkproj_gen(2, use_big_psum=False))))
        attn_emit(2, Fill(primed(qkproj_gen(3, use_big_psum=False))))

        # wo reuses wq's slot (wq dead after pair-3 projections)
        wo_sb = shared.tile([P, NPAIR, D], F16, name="wo_sb", tag="wq_wo")
        for o4 in range(NPAIR):
            nc.sync.dma_start(out=wo_sb[:, o4, :], in_=wor[:, o4, :])

        # pair 3 with the output projection interleaved per finished chunk
        attn_emit(3, None, after_chunk=outproj_chunk)


def build():
    nc = bacc.Bacc("TRN2", target_bir_lowering=False, debug=False, num_devices=8)
    xT_d = nc.dram_tensor("xT", [D, N], F16, kind="ExternalInput").ap()
    wq_d = nc.dram_tensor("wq", [D, DLOC], F16, kind="ExternalInput").ap()
    wk_d = nc.dram_tensor("wk", [D, DLOC], F16, kind="ExternalInput").ap()
    wv_d = nc.dram_tensor("wv", [D, DLOC], F16, kind="ExternalInput").ap()
    wo_d = nc.dram_tensor("wo", [DLOC, D], F16, kind="ExternalInput").ap()
    bq_d = nc.dram_tensor("bq", [P, NPAIR], F32, kind="ExternalInput").ap()
    bk_d = nc.dram_tensor("bk", [P, NPAIR], F32, kind="ExternalInput").ap()
    bv_d = nc.dram_tensor("bv", [P, NPAIR], F32, kind="ExternalInput").ap()
    out_d = nc.dram_tensor("out", [N, D], F32, kind="ExternalOutput").ap()
    with tile.TileContext(nc) as tc:
        _emit_kernel(tc, xT_d, wq_d, wk_d, wv_d, wo_d, bq_d, bk_d, bv_d, out_d)
    nc.compile()
    return nc


_NC = None


def _get_nc():
    global _NC
    if _NC is None:
        _NC = build()
    return _NC


def make_in_maps(x, Wq, bq, Wkv, bkv, Wo, bo):
    x = np.asarray(x, dtype=np.float32)
    Wq = np.asarray(Wq, dtype=np.float32)
    bq = np.asarray(bq, dtype=np.float32)
    Wkv = np.asarray(Wkv, dtype=np.float32)
    bkv = np.asarray(bkv, dtype=np.float32)
    Wo = np.asarray(Wo, dtype=np.float32)

    in_maps = []
    for c in range(8):
        bi, hj = c // 2, c % 2
        sl = slice(hj * DLOC, (hj + 1) * DLOC)
        slv = slice(D + hj * DLOC, D + (hj + 1) * DLOC)
        in_maps.append(
            {
                "xT": np.ascontiguousarray(x[bi].T).astype(np.float16),
                "wq": np.ascontiguousarray(Wq[:, sl]).astype(np.float16),
                "wk": np.ascontiguousarray(Wkv[:, sl]).astype(np.float16),
                "wv": np.ascontiguousarray(Wkv[:, slv]).astype(np.float16),
                "wo": np.ascontiguousarray(Wo[sl, :]).astype(np.float16),
                "bq": np.ascontiguousarray(bq[sl].reshape(NPAIR, P).T),
                "bk": np.ascontiguousarray(bkv[sl].reshape(NPAIR, P).T),
                "bv": np.ascontiguousarray(bkv[slv].reshape(NPAIR, P).T),
            }
        )
    return in_maps


def combine_outputs(results, bo):
    bo = np.asarray(bo, dtype=np.float32)
    outs = [results[c]["out"] for c in range(8)]
    full = np.stack([outs[2 * bi] + outs[2 * bi + 1] for bi in range(4)])
    return (full + bo[None, None, :]).astype(np.float32)


def kernel(x, Wq, bq, Wkv, bkv, Wo, bo, **_ignored):
    nc = _get_nc()
    in_maps = make_in_maps(x, Wq, bq, Wkv, bkv, Wo, bo)
    res = bass_utils.run_bass_kernel_spmd(nc, in_maps, core_ids=list(range(8)))
    return combine_outputs(res.results, bo)


# revision 33
# speedup vs baseline: 1.0496x; 1.0044x over previous
"""Trainium2 Bass kernel for causal multi-head attention.

Problem: x[4, 2048, 1024] -> Attention(heads=16, causal) -> out[4, 2048, 1024]

Sharding over 8 NeuronCores: core c handles batch bi = c // 2 and head-half
hj = c % 2 (8 of the 16 heads).  Each core computes its 8 heads' attention
and a partial output projection (row-parallel Wo); the host sums the two
partials per batch element and adds bo (the all-reduce step).

Per-core kernel (n=2048 tokens, dloc=512 local features, dh=64, 8 heads):
  - Host supplies x^T in fp16 (contraction dim on SBUF partitions, no
    on-device transposes).
  - Q^T, K^T [128 feats (pair of heads), 2048] per head-pair; V [2048, 512]
    in natural layout with a ones-column per head (V' = [V | 1]) so the
    PV matmul accumulates softmax denominators for free.
  - Scores are computed transposed: S^T[j, i] = k_j . q_i with K^T slices
    as the stationary operand.  K = dh = 64, and the two heads of a pair
    live at partition bases 0 / 64, which maps to PE row-groups 0-1 / 2-3:
    the hardware runs the pair concurrently (row packing).
  - exp on ScalarE straight out of PSUM (scale = 1/8 fused into the
    activation); causal mask applied post-exp by gpsimd affine_select
    (fill 0) on diagonal tiles only.
  - O'^T[f, i] accumulated over j-tiles in PSUM with lhsT = V'; row 64 is
    the softmax denominator.  O' is immediately staged PSUM->SBUF (frees
    the accumulator bank), then normalized off the critical path:
    1/sum via ln -> exp(-x) on ScalarE, broadcast down partitions with a
    K=1 outer-product matmul, multiply + bias on VectorE.
  - Output projection contracts the 512 local features from O^T directly.

All matmul operand tensors are float16 (fp32 accumulation in PSUM).  fp16
streams at 1 cycle/row like bf16 and allows standalone LDWEIGHTS (fp32/f32r
matmuls embed the weight load and serialize it, ~+150ns per matmul), but
keeps an 11-bit mantissa: end-to-end error vs the fp32 reference is ~5e-4.

Softmax reciprocals: the 1024 per-chunk denominators are DMA-repacked onto
128 partitions, reciprocal'd in one cheap DVE op ([1, N] DVE reciprocal is
~6.4 cyc/elem/lane, i.e. ~3.3us per row), DMA'd back to a partition-0 row
and broadcast down 64 partitions with the gpsimd partition_broadcast custom
instruction (which reads physical partition 0).
"""

import os
import sys

for _p in ("/opt/trn_rl_repo",):
    if _p not in sys.path and os.path.isdir(_p):
        sys.path.insert(0, _p)

import numpy as np

import concourse.bass as bass
import concourse.mybir as mybir
import concourse.tile as tile
from concourse import bacc
from concourse import bass_utils

ts = bass.ts
F32 = mybir.dt.float32
F16 = mybir.dt.float16

P = 128          # SBUF partitions
N = 2048         # sequence length
D = 1024         # model dim
DLOC = 512       # local (per-core) feature dim = 8 heads * 64
DH = 64          # head dim
NPAIR = 4        # head pairs per core (2 heads per pair = 128 feats)
NCO = D // P     # 8 contraction tiles over model dim
NJT = N // P     # 16 key tiles of 128
NCH = N // 512   # 4 query chunks of 512
SCALE = DH ** -0.5


def _emit_kernel(tc, xT_d, wq_d, wk_d, wv_d, wo_d, bq_d, bk_d, bv_d, out_d):
    nc = tc.nc
    EXP = mybir.ActivationFunctionType.Exp
    GE = mybir.AluOpType.is_ge

    xTr = xT_d.rearrange("(o p) t -> p o t", p=P)
    wqr = wq_d.rearrange("(o p) f -> p o f", p=P)
    wkr = wk_d.rearrange("(o p) f -> p o f", p=P)
    wvr = wv_d.rearrange("(o p) f -> p o f", p=P)
    wor = wo_d.rearrange("(o p) e -> p o e", p=P)

    with (
        nc.allow_low_precision(reason="fp16 operands / fp32 accumulation"),
        tc.tile_pool(name="perm", bufs=1) as perm,
        tc.tile_pool(name="shared", bufs=1) as shared,
        tc.tile_pool(name="qkt", bufs=3) as qktp,
        tc.tile_pool(name="pexp", bufs=6) as pexp,
        tc.tile_pool(name="stg", bufs=4) as stgp,
        tc.tile_pool(name="rrp", bufs=3) as rrp,
        tc.tile_pool(name="outsb", bufs=3) as outsb,
        tc.tile_pool(name="psS", bufs=2, space="PSUM") as psS,
        tc.tile_pool(name="psO", bufs=1, space="PSUM") as psO,
        tc.tile_pool(name="psProj", bufs=2, space="PSUM") as psProj,
    ):
        # ---- constants / weights ----------------------------------------
        bq_sb = perm.tile([P, NPAIR], F32, name="bq_sb")
        bk_sb = perm.tile([P, NPAIR], F32, name="bk_sb")
        bv_sb = perm.tile([P, NPAIR], F32, name="bv_sb")
        nc.sync.dma_start(out=bq_sb, in_=bq_d)
        nc.sync.dma_start(out=bk_sb, in_=bk_d)
        nc.sync.dma_start(out=bv_sb, in_=bv_d)

        # V' = [V | 1] per head: [128 j, jt, head, 65] fp16 (fp16 memset is
        # ISA-legal, unlike f32r; a broadcast DMA here would be 16K 2-byte
        # descriptors = ~150us of queue serialization)
        Vp = perm.tile([P, NJT, 8, DH + 1], F16, name="Vp")
        nc.vector.memset(Vp[:, :, :, DH:], 1.0)

        # DMA order matters: the first projection matmuls need wq + xT chunk
        # 0, so issue those first and the rest behind them.
        xT_sb = perm.tile([P, NCO, N], F16, name="xT_sb")
        wq_sb = shared.tile([P, NCO, DLOC], F16, name="wq_sb", tag="wq_wo")
        wk_sb = shared.tile([P, NCO, DLOC], F16, name="wk_sb", tag="wk")
        wv_sb = shared.tile([P, NCO, DLOC], F16, name="wv_sb", tag="wv_ot")
        for co in range(NCO):
            nc.sync.dma_start(out=wq_sb[:, co, :], in_=wqr[:, co, :])
            nc.sync.dma_start(
                out=xT_sb[:, co, ts(0, 512)], in_=xTr[:, co, ts(0, 512)]
            )
        for ch in range(1, NCH):
            for co in range(NCO):
                nc.sync.dma_start(
                    out=xT_sb[:, co, ts(ch, 512)], in_=xTr[:, co, ts(ch, 512)]
                )
        for co in range(NCO):
            nc.sync.dma_start(out=wk_sb[:, co, :], in_=wkr[:, co, :])
        for co in range(NCO):
            nc.sync.dma_start(out=wv_sb[:, co, :], in_=wvr[:, co, :])

        qk_tiles = {}

        def qkproj_gen(pair, use_big_psum):
            """Emit Q^T / K^T projection for one head pair; yields between ops."""
            QT = qktp.tile([P, N], F16, name=f"QT{pair}", tag="qt")
            KT = qktp.tile([P, N], F16, name=f"KT{pair}", tag="kt")
            qk_tiles[pair] = (QT, KT)
            for wsb, dst, bias in ((wq_sb, QT, bq_sb), (wk_sb, KT, bk_sb)):
                for ch in range(NCH):
                    if use_big_psum:
                        grp = psS.tile([P, 2, 512], F32, name="pj", tag="sg")
                        acc = grp[:, 0, :]
                    else:
                        acc = psProj.tile([P, 512], F32, name="pj", tag="proj")
                    for co in range(NCO):
                        nc.tensor.matmul(
                            acc,
                            lhsT=wsb[:, co, ts(pair, P)],
                            rhs=xT_sb[:, co, ts(ch, 512)],
                            start=(co == 0),
                            stop=(co == NCO - 1),
                        )
                        yield
                    nc.vector.tensor_scalar_add(
                        out=dst[:, ts(ch, 512)],
                        in0=acc,
                        scalar1=bias[:, pair : pair + 1],
                    )
                    yield "end"

        def vproj_gen(jt0, jt1, use_big_psum):
            for jt in range(jt0, jt1):
                if use_big_psum:
                    grp = psS.tile([P, 2, 512], F32, name="vps", tag="sg")
                    acc = grp[:, 0, :]
                else:
                    acc = psProj.tile([P, 512], F32, name="vps", tag="proj")
                for co in range(NCO):
                    nc.tensor.matmul(
                        acc,
                        lhsT=xT_sb[:, co, ts(jt, P)],
                        rhs=wv_sb[:, co, :],
                        start=(co == 0),
                        stop=(co == NCO - 1),
                    )
                    yield
                nc.vector.tensor_copy(
                    out=Vp[:, jt, :, 0:DH],
                    in_=acc.rearrange("p (h f) -> p h f", h=8),
                )
                yield "end"

        def chain(*gens):
            for g in gens:
                yield from g

        class Fill:
            """Dispenses filler ops; a PSUM-accumulator group must never
            straddle an attention chunk boundary (its DVE eviction would
            queue behind the next chunk's PV start while PV waits on the
            slot that eviction frees -> deadlock)."""

            def __init__(self, gen):
                self.gen = gen
                self.in_group = False

            def _next(self):
                v = next(self.gen, StopIteration)
                if v is StopIteration:
                    self.gen = None
                    self.in_group = False
                    return False
                self.in_group = v != "end"
                return True

            def pull(self, n):
                for _ in range(n):
                    if self.gen is None or not self._next():
                        return

            def finish_group(self):
                while self.gen is not None and self.in_group:
                    self._next()

            def drain(self):
                while self.gen is not None and self._next():
                    pass

        def attn_emit(pair, fill, after_chunk=None):
            QT, KT = qk_tiles[pair]
            hA, hB = 2 * pair, 2 * pair + 1
            # big chunk first: the last chunk's normalize chain then hides
            # under the previous chunk's output-projection matmuls
            chunk_order = list(range(NCH - 1, -1, -1))
            for chi, ch in enumerate(chunk_order):
                if after_chunk is not None and chi > 0:
                    after_chunk(chunk_order[chi - 1])
                oA = psO.tile([P, 512], F32, name="oA", tag="oA")
                oB = psO.tile([P, 512], F32, name="oB", tag="oB")
                njt = 4 * ch + 4
                pend = []

                def pv(pt, jt, njt=njt, oA=oA, oB=oB, hA=hA, hB=hB, ch=ch):
                    # below-diagonal columns of pt are all-zero: skip them
                    # (they contribute nothing; has_written tracking is
                    # per-element so partial-width accumulation is fine, and
                    # jt==0 always writes the full width)
                    plo = P * (jt - 4 * ch) if jt - 4 * ch > 0 else 0
                    for h01, (oP, h) in enumerate(((oA, hA), (oB, hB))):
                        nc.tensor.matmul(
                            oP[0 : DH + 1, plo:512],
                            lhsT=Vp[:, jt, h, :],
                            rhs=pt[:, h01, plo:512],
                            start=(jt == 0),
                            stop=(jt == njt - 1),
                        )

                for jt in range(njt):
                    if fill is not None:
                        fill.pull(2)
                    sg = psS.tile([P, 2, 512], F32, name="sg", tag="sg")
                    # diagonal j-tiles: columns q < 128*r are entirely below
                    # the causal diagonal; skip computing them (the masking
                    # affine_select fills that region of pt with 0 anyway,
                    # covering the garbage left in PSUM)
                    r0 = jt - 4 * ch
                    lo = P * r0 if r0 > 0 else 0
                    nc.tensor.matmul(
                        sg[:, 0, lo:512],
                        lhsT=KT[0:DH, ts(jt, P)],
                        rhs=QT[0:DH, 512 * ch + lo : 512 * (ch + 1)],
                        start=True,
                        stop=True,
                    )
                    nc.tensor.matmul(
                        sg[:, 1, lo:512],
                        lhsT=KT[DH:P, ts(jt, P)],
                        rhs=QT[DH:P, 512 * ch + lo : 512 * (ch + 1)],
                        start=True,
                        stop=True,
                    )
                    pt = pexp.tile([P, 2, 512], F16, name="pt", tag="pt")
                    nc.scalar.activation(
                        out=pt[:, :, lo:512], in_=sg[:, :, lo:512], func=EXP,
                        scale=SCALE,
                    )
                    r = jt - 4 * ch
                    if r >= 0:
                        if lo > 0:
                            # columns entirely below the diagonal: never
                            # computed, but read by the PV matmul -> zero them
                            nc.gpsimd.memset(pt[:, :, 0:lo], 0.0)
                        for h01 in (0, 1):
                            # keep where q >= 128*r + p  (j_global <= i_global)
                            nc.gpsimd.affine_select(
                                out=pt[:, h01, lo : lo + P],
                                in_=pt[:, h01, lo : lo + P],
                                compare_op=GE,
                                fill=0.0,
                                base=0,
                                channel_multiplier=-1,
                                pattern=[[1, P]],
                            )
                    pend.append((pt, jt))
                    if len(pend) > 2:
                        pv(*pend.pop(0))
                for args in pend:
                    pv(*args)
                if fill is not None:
                    fill.finish_group()

                # ---- stage O' out of PSUM, then normalize off-path --------
                st = stgp.tile([DH + 1, 2, 512], F32, name="st", tag="st")
                nc.vector.tensor_copy(out=st[:, 0, :], in_=oA[0 : DH + 1, :])
                nc.vector.tensor_copy(out=st[:, 1, :], in_=oB[0 : DH + 1, :])
                # Reciprocal of the 1024 sums at full DVE lane utilization:
                # DVE recip costs ~6.4 cyc/elem PER LANE, so [1, 1024] would
                # be ~7us.  Pack the sums onto 128 partitions via DMA
                # (element-wise stream repack), recip [128, 8] (~0.1us),
                # unpack to a partition-0 row, then gpsimd-broadcast it down
                # 64 partitions (partition_broadcast reads physical p0).
                pk = rrp.tile([P, 8], F32, name="pk", tag="pk")
                rrow = rrp.tile([1, 2, 512], F32, name="rrow", tag="rrow")
                Rs = rrp.tile([DH, 2, 512], F32, name="Rs", tag="Rs")
                nc.sync.dma_start(out=pk, in_=st[DH : DH + 1, :, :])
                nc.vector.reciprocal(out=pk, in_=pk)
                nc.sync.dma_start(out=rrow, in_=pk)
                for h01 in (0, 1):
                    nc.gpsimd.partition_broadcast(Rs[:, h01, :], rrow[0:1, h01, :])
                nc.vector.tensor_mul(
                    out=OT[0:DH, pair, ts(ch, 512)], in0=st[0:DH, 0, :], in1=Rs[:, 0, :]
                )
                nc.vector.tensor_scalar_add(
                    out=OT[0:DH, pair, ts(ch, 512)],
                    in0=OT[0:DH, pair, ts(ch, 512)],
                    scalar1=bv_sb[0:DH, pair : pair + 1],
                )
                nc.vector.tensor_mul(
                    out=OT[DH:P, pair, ts(ch, 512)], in0=st[0:DH, 1, :], in1=Rs[:, 1, :]
                )
                nc.vector.tensor_scalar_add(
                    out=OT[DH:P, pair, ts(ch, 512)],
                    in0=OT[DH:P, pair, ts(ch, 512)],
                    scalar1=bv_sb[DH:P, pair : pair + 1],
                )
                if after_chunk is not None and chi == NCH - 1:
                    after_chunk(ch)
            if fill is not None:
                fill.drain()

        def outproj_chunk(ch):
            for it in range(4 * ch, 4 * ch + 4):
                for e in range(2):
                    acc = psProj.tile([P, 512], F32, name="ops", tag="proj")
                    for p4 in range(NPAIR):
                        nc.tensor.matmul(
                            acc,
                            lhsT=OT[:, p4, ts(it, P)],
                            rhs=wo_sb[:, p4, ts(e, 512)],
                            start=(p4 == 0),
                            stop=(p4 == NPAIR - 1),
                        )
                    ob = outsb.tile([P, 512], F32, name="ob", tag="ob")
                    nc.vector.tensor_copy(out=ob, in_=acc)
                    nc.sync.dma_start(out=out_d[ts(it, P), ts(e, 512)], in_=ob)

        # ---- emission schedule ------------------------------------------
        # upfront: QK projections for pair 0 + all of V (attention consumes
        # V j-tiles far faster than a fill stream could produce them)
        for _ in qkproj_gen(0, use_big_psum=True):
            pass
        for _ in vproj_gen(0, NJT, use_big_psum=True):
            pass

        # OT reuses wv's slot (wv dead once V is projected)
        OT = shared.tile([P, NPAIR, N], F16, name="OT", tag="wv_ot")

        def primed(gen):
            next(gen)
            return gen

        # attention for pair p overlapped with projections for pair p+1
        attn_emit(0, Fill(primed(qkproj_gen(1, use_big_psum=False))))
        attn_emit(1, Fill(primed(qkproj_gen(2, use_big_psum=False))))
        attn_emit(2, Fill(primed(qkproj_gen(3, use_big_psum=False))))

        # wo reuses wq's slot (wq dead after pair-3 projections)
        wo_sb = shared.tile([P, NPAIR, D], F16, name="wo_sb", tag="wq_wo")
        for o4 in range(NPAIR):
            nc.sync.dma_start(out=wo_sb[:, o4, :], in_=wor[:, o4, :])

        # pair 3 with the output projection interleaved per finished chunk
        attn_emit(3, None, after_chunk=outproj_chunk)


def build():
    nc = bacc.Bacc("TRN2", target_bir_lowering=False, debug=False, num_devices=8)
    xT_d = nc.dram_tensor("xT", [D, N], F16, kind="ExternalInput").ap()
    wq_d = nc.dram_tensor("wq", [D, DLOC], F16, kind="ExternalInput").ap()
    wk_d = nc.dram_tensor("wk", [D, DLOC], F16, kind="ExternalInput").ap()
    wv_d = nc.dram_tensor("wv", [D, DLOC], F16, kind="ExternalInput").ap()
    wo_d = nc.dram_tensor("wo", [DLOC, D], F16, kind="ExternalInput").ap()
    bq_d = nc.dram_tensor("bq", [P, NPAIR], F32, kind="ExternalInput").ap()
    bk_d = nc.dram_tensor("bk", [P, NPAIR], F32, kind="ExternalInput").ap()
    bv_d = nc.dram_tensor("bv", [P, NPAIR], F32, kind="ExternalInput").ap()
    out_d = nc.dram_tensor("out", [N, D], F32, kind="ExternalOutput").ap()
    with tile.TileContext(nc) as tc:
        _emit_kernel(tc, xT_d, wq_d, wk_d, wv_d, wo_d, bq_d, bk_d, bv_d, out_d)
    nc.compile()
    return nc


_NC = None


def _get_nc():
    global _NC
    if _NC is None:
        _NC = build()
    return _NC


def make_in_maps(x, Wq, bq, Wkv, bkv, Wo, bo):
    x = np.asarray(x, dtype=np.float32)
    Wq = np.asarray(Wq, dtype=np.float32)
    bq = np.asarray(bq, dtype=np.float32)
    Wkv = np.asarray(Wkv, dtype=np.float32)
    bkv = np.asarray(bkv, dtype=np.float32)
    Wo = np.asarray(Wo, dtype=np.float32)

    in_maps = []
    for c in range(8):
        bi, hj = c // 2, c % 2
        sl = slice(hj * DLOC, (hj + 1) * DLOC)
        slv = slice(D + hj * DLOC, D + (hj + 1) * DLOC)
        in_maps.append(
            {
                "xT": np.ascontiguousarray(x[bi].T).astype(np.float16),
                "wq": np.ascontiguousarray(Wq[:, sl]).astype(np.float16),
                "wk": np.ascontiguousarray(Wkv[:, sl]).astype(np.float16),
                "wv": np.ascontiguousarray(Wkv[:, slv]).astype(np.float16),
                "wo": np.ascontiguousarray(Wo[sl, :]).astype(np.float16),
                "bq": np.ascontiguousarray(bq[sl].reshape(NPAIR, P).T),
                "bk": np.ascontiguousarray(bkv[sl].reshape(NPAIR, P).T),
                "bv": np.ascontiguousarray(bkv[slv].reshape(NPAIR, P).T),
            }
        )
    return in_maps


def combine_outputs(results, bo):
    bo = np.asarray(bo, dtype=np.float32)
    outs = [results[c]["out"] for c in range(8)]
    full = np.stack([outs[2 * bi] + outs[2 * bi + 1] for bi in range(4)])
    return (full + bo[None, None, :]).astype(np.float32)


def kernel(x, Wq, bq, Wkv, bkv, Wo, bo, **_ignored):
    nc = _get_nc()
    in_maps = make_in_maps(x, Wq, bq, Wkv, bkv, Wo, bo)
    res = bass_utils.run_bass_kernel_spmd(nc, in_maps, core_ids=list(range(8)))
    return combine_outputs(res.results, bo)


# revision 35
# speedup vs baseline: 1.0640x; 1.0137x over previous
"""Trainium2 Bass kernel for causal multi-head attention.

Problem: x[4, 2048, 1024] -> Attention(heads=16, causal) -> out[4, 2048, 1024]

Sharding over 8 NeuronCores: core c handles batch bi = c // 2 and head-half
hj = c % 2 (8 of the 16 heads).  Each core computes its 8 heads' attention
and a partial output projection (row-parallel Wo); the host sums the two
partials per batch element and adds bo (the all-reduce step).

Per-core kernel (n=2048 tokens, dloc=512 local features, dh=64, 8 heads):
  - Host supplies x^T in fp16 (contraction dim on SBUF partitions, no
    on-device transposes).
  - Q^T, K^T [128 feats (pair of heads), 2048] per head-pair; V [2048, 512]
    in natural layout with a ones-column per head (V' = [V | 1]) so the
    PV matmul accumulates softmax denominators for free.
  - Scores are computed transposed: S^T[j, i] = k_j . q_i with K^T slices
    as the stationary operand.  K = dh = 64, and the two heads of a pair
    live at partition bases 0 / 64, which maps to PE row-groups 0-1 / 2-3:
    the hardware runs the pair concurrently (row packing).
  - exp on ScalarE straight out of PSUM (scale = 1/8 fused into the
    activation); causal mask applied post-exp by gpsimd affine_select
    (fill 0) on diagonal tiles only.
  - O'^T[f, i] accumulated over j-tiles in PSUM with lhsT = V'; row 64 is
    the softmax denominator.  O' is immediately staged PSUM->SBUF (frees
    the accumulator bank), then normalized off the critical path:
    1/sum via ln -> exp(-x) on ScalarE, broadcast down partitions with a
    K=1 outer-product matmul, multiply + bias on VectorE.
  - Output projection contracts the 512 local features from O^T directly.

All matmul operand tensors are float16 (fp32 accumulation in PSUM).  fp16
streams at 1 cycle/row like bf16 and allows standalone LDWEIGHTS (fp32/f32r
matmuls embed the weight load and serialize it, ~+150ns per matmul), but
keeps an 11-bit mantissa: end-to-end error vs the fp32 reference is ~5e-4.

Softmax reciprocals: the 1024 per-chunk denominators are DMA-repacked onto
128 partitions, reciprocal'd in one cheap DVE op ([1, N] DVE reciprocal is
~6.4 cyc/elem/lane, i.e. ~3.3us per row), DMA'd back to a partition-0 row
and broadcast down 64 partitions with the gpsimd partition_broadcast custom
instruction (which reads physical partition 0).
"""

import os
import sys

for _p in ("/opt/trn_rl_repo",):
    if _p not in sys.path and os.path.isdir(_p):
        sys.path.insert(0, _p)

import numpy as np

import concourse.bass as bass
import concourse.mybir as mybir
import concourse.tile as tile
from concourse import bacc
from concourse import bass_utils

ts = bass.ts
F32 = mybir.dt.float32
F16 = mybir.dt.float16

P = 128          # SBUF partitions
N = 2048         # sequence length
D = 1024         # model dim
DLOC = 512       # local (per-core) feature dim = 8 heads * 64
DH = 64          # head dim
NPAIR = 4        # head pairs per core (2 heads per pair = 128 feats)
NCO = D // P     # 8 contraction tiles over model dim
NJT = N // P     # 16 key tiles of 128
NCH = N // 512   # 4 query chunks of 512
SCALE = DH ** -0.5


def _emit_kernel(tc, xT_d, wq_d, wk_d, wv_d, wo_d, bq_d, bk_d, bv_d, out_d):
    nc = tc.nc
    EXP = mybir.ActivationFunctionType.Exp
    GE = mybir.AluOpType.is_ge

    xTr = xT_d.rearrange("(o p) t -> p o t", p=P)
    wqr = wq_d.rearrange("(o p) f -> p o f", p=P)
    wkr = wk_d.rearrange("(o p) f -> p o f", p=P)
    wvr = wv_d.rearrange("(o p) f -> p o f", p=P)
    wor = wo_d.rearrange("(o p) e -> p o e", p=P)

    with (
        nc.allow_low_precision(reason="fp16 operands / fp32 accumulation"),
        tc.tile_pool(name="perm", bufs=1) as perm,
        tc.tile_pool(name="shared", bufs=1) as shared,
        tc.tile_pool(name="qkt", bufs=3) as qktp,
        tc.tile_pool(name="pexp", bufs=6) as pexp,
        tc.tile_pool(name="stg", bufs=4) as stgp,
        tc.tile_pool(name="rrp", bufs=3) as rrp,
        tc.tile_pool(name="outsb", bufs=3) as outsb,
        tc.tile_pool(name="psS", bufs=2, space="PSUM") as psS,
        tc.tile_pool(name="psO", bufs=1, space="PSUM") as psO,
        tc.tile_pool(name="psProj", bufs=2, space="PSUM") as psProj,
    ):
        # ---- constants / weights ----------------------------------------
        bq_sb = perm.tile([P, NPAIR], F32, name="bq_sb")
        bk_sb = perm.tile([P, NPAIR], F32, name="bk_sb")
        bv_sb = perm.tile([P, NPAIR], F32, name="bv_sb")
        nc.sync.dma_start(out=bq_sb, in_=bq_d)
        nc.sync.dma_start(out=bk_sb, in_=bk_d)
        nc.sync.dma_start(out=bv_sb, in_=bv_d)

        # V' = [V | 1] per head: [128 j, jt, head, 65] fp16 (fp16 memset is
        # ISA-legal, unlike f32r; a broadcast DMA here would be 16K 2-byte
        # descriptors = ~150us of queue serialization)
        Vp = perm.tile([P, NJT, 8, DH + 1], F16, name="Vp")
        nc.vector.memset(Vp[:, :, :, DH:], 1.0)

        # DMA order matters: the first projection matmuls need wq + xT chunk
        # 0, so issue those first and the rest behind them.
        xT_sb = perm.tile([P, NCO, N], F16, name="xT_sb")
        wq_sb = shared.tile([P, NCO, DLOC], F16, name="wq_sb", tag="wq_wo")
        wk_sb = shared.tile([P, NCO, DLOC], F16, name="wk_sb", tag="wk")
        wv_sb = shared.tile([P, NCO, DLOC], F16, name="wv_sb", tag="wv_ot")
        for co in range(NCO):
            nc.sync.dma_start(out=wq_sb[:, co, :], in_=wqr[:, co, :])
            nc.sync.dma_start(
                out=xT_sb[:, co, ts(0, 512)], in_=xTr[:, co, ts(0, 512)]
            )
        for ch in range(1, NCH):
            for co in range(NCO):
                nc.sync.dma_start(
                    out=xT_sb[:, co, ts(ch, 512)], in_=xTr[:, co, ts(ch, 512)]
                )
        for co in range(NCO):
            nc.sync.dma_start(out=wk_sb[:, co, :], in_=wkr[:, co, :])
        for co in range(NCO):
            nc.sync.dma_start(out=wv_sb[:, co, :], in_=wvr[:, co, :])

        qk_tiles = {}

        def qkproj_gen(pair, use_big_psum):
            """Emit Q^T / K^T projection for one head pair; yields between ops."""
            QT = qktp.tile([P, N], F16, name=f"QT{pair}", tag="qt")
            KT = qktp.tile([P, N], F16, name=f"KT{pair}", tag="kt")
            qk_tiles[pair] = (QT, KT)
            for wsb, dst, bias in ((wq_sb, QT, bq_sb), (wk_sb, KT, bk_sb)):
                for ch in range(NCH):
                    if use_big_psum:
                        grp = psS.tile([P, 2, 512], F32, name="pj", tag="sg")
                        acc = grp[:, 0, :]
                    else:
                        acc = psProj.tile([P, 512], F32, name="pj", tag="proj")
                    for co in range(NCO):
                        nc.tensor.matmul(
                            acc,
                            lhsT=wsb[:, co, ts(pair, P)],
                            rhs=xT_sb[:, co, ts(ch, 512)],
                            start=(co == 0),
                            stop=(co == NCO - 1),
                        )
                        yield
                    # eviction + bias fused on ScalarE (ACT has slack; a
                    # shorter DVE queue frees PSUM accumulators faster)
                    nc.scalar.activation(
                        out=dst[:, ts(ch, 512)],
                        in_=acc,
                        func=mybir.ActivationFunctionType.Identity,
                        bias=bias[:, pair : pair + 1],
                    )
                    yield "end"

        def vproj_gen(jt0, jt1, use_big_psum):
            for jt in range(jt0, jt1):
                if use_big_psum:
                    grp = psS.tile([P, 2, 512], F32, name="vps", tag="sg")
                    acc = grp[:, 0, :]
                else:
                    acc = psProj.tile([P, 512], F32, name="vps", tag="proj")
                for co in range(NCO):
                    nc.tensor.matmul(
                        acc,
                        lhsT=xT_sb[:, co, ts(jt, P)],
                        rhs=wv_sb[:, co, :],
                        start=(co == 0),
                        stop=(co == NCO - 1),
                    )
                    yield
                nc.scalar.activation(
                    out=Vp[:, jt, :, 0:DH],
                    in_=acc.rearrange("p (h f) -> p h f", h=8),
                    func=mybir.ActivationFunctionType.Copy,
                )
                yield "end"

        def chain(*gens):
            for g in gens:
                yield from g

        class Fill:
            """Dispenses filler ops; a PSUM-accumulator group must never
            straddle an attention chunk boundary (its DVE eviction would
            queue behind the next chunk's PV start while PV waits on the
            slot that eviction frees -> deadlock)."""

            def __init__(self, gen):
                self.gen = gen
                self.in_group = False

            def _next(self):
                v = next(self.gen, StopIteration)
                if v is StopIteration:
                    self.gen = None
                    self.in_group = False
                    return False
                self.in_group = v != "end"
                return True

            def pull(self, n):
                for _ in range(n):
                    if self.gen is None or not self._next():
                        return

            def finish_group(self):
                while self.gen is not None and self.in_group:
                    self._next()

            def drain(self):
                while self.gen is not None and self._next():
                    pass

        def attn_emit(pair, fill, after_chunk=None):
            QT, KT = qk_tiles[pair]
            hA, hB = 2 * pair, 2 * pair + 1
            # big chunk first: the last chunk's normalize chain then hides
            # under the previous chunk's output-projection matmuls
            chunk_order = list(range(NCH - 1, -1, -1))
            for chi, ch in enumerate(chunk_order):
                if after_chunk is not None and chi > 0:
                    after_chunk(chunk_order[chi - 1])
                oA = psO.tile([P, 512], F32, name="oA", tag="oA")
                oB = psO.tile([P, 512], F32, name="oB", tag="oB")
                njt = 4 * ch + 4
                pend = []

                def pv(pt, jt, njt=njt, oA=oA, oB=oB, hA=hA, hB=hB, ch=ch):
                    # below-diagonal columns of pt are all-zero: skip them
                    # (they contribute nothing; has_written tracking is
                    # per-element so partial-width accumulation is fine, and
                    # jt==0 always writes the full width)
                    plo = P * (jt - 4 * ch) if jt - 4 * ch > 0 else 0
                    for h01, (oP, h) in enumerate(((oA, hA), (oB, hB))):
                        nc.tensor.matmul(
                            oP[0 : DH + 1, plo:512],
                            lhsT=Vp[:, jt, h, :],
                            rhs=pt[:, h01, plo:512],
                            start=(jt == 0),
                            stop=(jt == njt - 1),
                        )

                for jt in range(njt):
                    if fill is not None:
                        fill.pull(2)
                    sg = psS.tile([P, 2, 512], F32, name="sg", tag="sg")
                    # diagonal j-tiles: columns q < 128*r are entirely below
                    # the causal diagonal; skip computing them (the masking
                    # affine_select fills that region of pt with 0 anyway,
                    # covering the garbage left in PSUM)
                    r0 = jt - 4 * ch
                    lo = P * r0 if r0 > 0 else 0
                    nc.tensor.matmul(
                        sg[:, 0, lo:512],
                        lhsT=KT[0:DH, ts(jt, P)],
                        rhs=QT[0:DH, 512 * ch + lo : 512 * (ch + 1)],
                        start=True,
                        stop=True,
                    )
                    nc.tensor.matmul(
                        sg[:, 1, lo:512],
                        lhsT=KT[DH:P, ts(jt, P)],
                        rhs=QT[DH:P, 512 * ch + lo : 512 * (ch + 1)],
                        start=True,
                        stop=True,
                    )
                    pt = pexp.tile([P, 2, 512], F16, name="pt", tag="pt")
                    nc.scalar.activation(
                        out=pt[:, :, lo:512], in_=sg[:, :, lo:512], func=EXP,
                        scale=SCALE,
                    )
                    r = jt - 4 * ch
                    if r >= 0:
                        if lo > 0:
                            # columns entirely below the diagonal: never
                            # computed, but read by the PV matmul -> zero them
                            nc.gpsimd.memset(pt[:, :, 0:lo], 0.0)
                        for h01 in (0, 1):
                            # keep where q >= 128*r + p  (j_global <= i_global)
                            nc.gpsimd.affine_select(
                                out=pt[:, h01, lo : lo + P],
                                in_=pt[:, h01, lo : lo + P],
                                compare_op=GE,
                                fill=0.0,
                                base=0,
                                channel_multiplier=-1,
                                pattern=[[1, P]],
                            )
                    pend.append((pt, jt))
                    if len(pend) > 2:
                        pv(*pend.pop(0))
                for args in pend:
                    pv(*args)
                if fill is not None:
                    fill.finish_group()

                # ---- stage O' out of PSUM, then normalize off-path --------
                st = stgp.tile([DH + 1, 2, 512], F32, name="st", tag="st")
                nc.vector.tensor_copy(out=st[:, 0, :], in_=oA[0 : DH + 1, :])
                nc.vector.tensor_copy(out=st[:, 1, :], in_=oB[0 : DH + 1, :])
                # Reciprocal of the 1024 sums at full DVE lane utilization:
                # DVE recip costs ~6.4 cyc/elem PER LANE, so [1, 1024] would
                # be ~7us.  Pack the sums onto 128 partitions via DMA
                # (element-wise stream repack), recip [128, 8] (~0.1us),
                # unpack to a partition-0 row, then gpsimd-broadcast it down
                # 64 partitions (partition_broadcast reads physical p0).
                pk = rrp.tile([P, 8], F32, name="pk", tag="pk")
                rrow = rrp.tile([1, 2, 512], F32, name="rrow", tag="rrow")
                Rs = rrp.tile([DH, 2, 512], F32, name="Rs", tag="Rs")
                nc.sync.dma_start(out=pk, in_=st[DH : DH + 1, :, :])
                nc.vector.reciprocal(out=pk, in_=pk)
                nc.sync.dma_start(out=rrow, in_=pk)
                for h01 in (0, 1):
                    nc.gpsimd.partition_broadcast(Rs[:, h01, :], rrow[0:1, h01, :])
                nc.vector.tensor_mul(
                    out=OT[0:DH, pair, ts(ch, 512)], in0=st[0:DH, 0, :], in1=Rs[:, 0, :]
                )
                nc.vector.tensor_scalar_add(
                    out=OT[0:DH, pair, ts(ch, 512)],
                    in0=OT[0:DH, pair, ts(ch, 512)],
                    scalar1=bv_sb[0:DH, pair : pair + 1],
                )
                nc.vector.tensor_mul(
                    out=OT[DH:P, pair, ts(ch, 512)], in0=st[0:DH, 1, :], in1=Rs[:, 1, :]
                )
                nc.vector.tensor_scalar_add(
                    out=OT[DH:P, pair, ts(ch, 512)],
                    in0=OT[DH:P, pair, ts(ch, 512)],
                    scalar1=bv_sb[DH:P, pair : pair + 1],
                )
                if after_chunk is not None and chi == NCH - 1:
                    after_chunk(ch)
            if fill is not None:
                fill.drain()

        def outproj_chunk(ch):
            for it in range(4 * ch, 4 * ch + 4):
                for e in range(2):
                    acc = psProj.tile([P, 512], F32, name="ops", tag="proj")
                    for p4 in range(NPAIR):
                        nc.tensor.matmul(
                            acc,
                            lhsT=OT[:, p4, ts(it, P)],
                            rhs=wo_sb[:, p4, ts(e, 512)],
                            start=(p4 == 0),
                            stop=(p4 == NPAIR - 1),
                        )
                    ob = outsb.tile([P, 512], F32, name="ob", tag="ob")
                    nc.vector.tensor_copy(out=ob, in_=acc)
                    nc.sync.dma_start(out=out_d[ts(it, P), ts(e, 512)], in_=ob)

        # ---- emission schedule ------------------------------------------
        # upfront: QK projections for pair 0 + all of V (attention consumes
        # V j-tiles far faster than a fill stream could produce them)
        for _ in qkproj_gen(0, use_big_psum=True):
            pass
        for _ in vproj_gen(0, NJT, use_big_psum=True):
            pass

        # OT reuses wv's slot (wv dead once V is projected)
        OT = shared.tile([P, NPAIR, N], F16, name="OT", tag="wv_ot")

        def primed(gen):
            next(gen)
            return gen

        # attention for pair p overlapped with projections for pair p+1
        attn_emit(0, Fill(primed(qkproj_gen(1, use_big_psum=False))))
        attn_emit(1, Fill(primed(qkproj_gen(2, use_big_psum=False))))
        attn_emit(2, Fill(primed(qkproj_gen(3, use_big_psum=False))))

        # wo reuses wq's slot (wq dead after pair-3 projections)
        wo_sb = shared.tile([P, NPAIR, D], F16, name="wo_sb", tag="wq_wo")
        for o4 in range(NPAIR):
            nc.sync.dma_start(out=wo_sb[:, o4, :], in_=wor[:, o4, :])

        # pair 3 with the output projection interleaved per finished chunk
        attn_emit(3, None, after_chunk=outproj_chunk)


def build():
    nc = bacc.Bacc("TRN2", target_bir_lowering=False, debug=False, num_devices=8)
    xT_d = nc.dram_tensor("xT", [D, N], F16, kind="ExternalInput").ap()
    wq_d = nc.dram_tensor("wq", [D, DLOC], F16, kind="ExternalInput").ap()
    wk_d = nc.dram_tensor("wk", [D, DLOC], F16, kind="ExternalInput").ap()
    wv_d = nc.dram_tensor("wv", [D, DLOC], F16, kind="ExternalInput").ap()
    wo_d = nc.dram_tensor("wo", [DLOC, D], F16, kind="ExternalInput").ap()
    bq_d = nc.dram_tensor("bq", [P, NPAIR], F32, kind="ExternalInput").ap()
    bk_d = nc.dram_tensor("bk", [P, NPAIR], F32, kind="ExternalInput").ap()
    bv_d = nc.dram_tensor("bv", [P, NPAIR], F32, kind="ExternalInput").ap()
    out_d = nc.dram_tensor("out", [N, D], F32, kind="ExternalOutput").ap()
    with tile.TileContext(nc) as tc:
        _emit_kernel(tc, xT_d, wq_d, wk_d, wv_d, wo_d, bq_d, bk_d, bv_d, out_d)
    nc.compile()
    return nc


_NC = None


def _get_nc():
    global _NC
    if _NC is None:
        _NC = build()
    return _NC


def make_in_maps(x, Wq, bq, Wkv, bkv, Wo, bo):
    x = np.asarray(x, dtype=np.float32)
    Wq = np.asarray(Wq, dtype=np.float32)
    bq = np.asarray(bq, dtype=np.float32)
    Wkv = np.asarray(Wkv, dtype=np.float32)
    bkv = np.asarray(bkv, dtype=np.float32)
    Wo = np.asarray(Wo, dtype=np.float32)

    in_maps = []
    for c in range(8):
        bi, hj = c // 2, c % 2
        sl = slice(hj * DLOC, (hj + 1) * DLOC)
        slv = slice(D + hj * DLOC, D + (hj + 1) * DLOC)
        in_maps.append(
            {
                "xT": np.ascontiguousarray(x[bi].T).astype(np.float16),
                "wq": np.ascontiguousarray(Wq[:, sl]).astype(np.float16),
                "wk": np.ascontiguousarray(Wkv[:, sl]).astype(np.float16),
                "wv": np.ascontiguousarray(Wkv[:, slv]).astype(np.float16),
                "wo": np.ascontiguousarray(Wo[sl, :]).astype(np.float16),
                "bq": np.ascontiguousarray(bq[sl].reshape(NPAIR, P).T),
                "bk": np.ascontiguousarray(bkv[sl].reshape(NPAIR, P).T),
                "bv": np.ascontiguousarray(bkv[slv].reshape(NPAIR, P).T),
            }
        )
    return in_maps


def combine_outputs(results, bo):
    bo = np.asarray(bo, dtype=np.float32)
    outs = [results[c]["out"] for c in range(8)]
    full = np.stack([outs[2 * bi] + outs[2 * bi + 1] for bi in range(4)])
    return (full + bo[None, None, :]).astype(np.float32)


def kernel(x, Wq, bq, Wkv, bkv, Wo, bo, **_ignored):
    nc = _get_nc()
    in_maps = make_in_maps(x, Wq, bq, Wkv, bkv, Wo, bo)
    res = bass_utils.run_bass_kernel_spmd(nc, in_maps, core_ids=list(range(8)))
    return combine_outputs(res.results, bo)
